# revision 17
# baseline (speedup 1.0000x reference)
"""Causal self-attention (B=4, T=2048, C=768, H=12) on 8 trn2 NeuronCores.

Sharding: core c -> batch c//2, head-group c%2 (6 heads each).
Each core computes qkv projection, flash-style causal attention and its
partial c_proj contribution for its 6 heads; the host sums the two
head-group partials per batch and adds b_proj.
"""

import numpy as np
import ml_dtypes

_BF16 = ml_dtypes.bfloat16

B, T, C = 4, 2048, 768
H, HD = 12, 64
NCORES = 8
NH = 6            # heads per core
CQ = NH * HD      # 384
CPAD = 768        # contraction dim (bias folded into copies)
TQ = 512          # query chunk
KCB = 128         # key block
SGRP = 2          # key blocks per PSUM S-group (2 banks)
VSTR = NH * (HD + 1)  # 390: v_sb stride per key block (65 per head)

_cache = {}


def _build():
    import concourse.bacc as bacc
    import concourse.bass as bass
    from concourse import mybir
    from concourse.tile import TileContext

    f32 = mybir.dt.float32
    bf16 = mybir.dt.bfloat16
    EXP = mybir.ActivationFunctionType.Exp

    nc = bacc.Bacc("TRN2", target_bir_lowering=False, debug=False)
    d_xt = nc.dram_tensor("xt", [CPAD, T], bf16, kind="ExternalInput")
    d_wqk = nc.dram_tensor("wqk", [CPAD, 2 * CQ], bf16, kind="ExternalInput")
    d_wv = nc.dram_tensor("wv", [CPAD, CQ], bf16, kind="ExternalInput")
    d_wp = nc.dram_tensor("wp", [CQ, C], bf16, kind="ExternalInput")
    d_mk = nc.dram_tensor("mk", [KCB, KCB], bf16, kind="ExternalInput")
    d_bqk = nc.dram_tensor("bqk", [128, 6], f32, kind="ExternalInput")
    d_bv = nc.dram_tensor("bv", [1, CQ], f32, kind="ExternalInput")
    d_out = nc.dram_tensor("out", [T, C], bf16, kind="ExternalOutput")

    NQC = T // TQ    # 4 query chunks
    NCC = CPAD // 128  # 7 contraction chunks

    with TileContext(nc) as tc:
        with tc.tile_pool(name="const", bufs=1) as const:
            # persistent SBUF tiles
            xt_sb = [const.tile([128, T], bf16, name=f"xt{i}", tag=f"xt{i}") for i in range(NCC)]
            wqk_sb = [const.tile([128, 2 * CQ], bf16, name=f"wqk{i}", tag=f"wqk{i}") for i in range(NCC)]
            wv_sb = [const.tile([128, CQ], bf16, name=f"wv{i}", tag=f"wv{i}") for i in range(NCC)]
            wp_sb = [const.tile([128, C], bf16, name=f"wp{i}", tag=f"wp{i}") for i in range(3)]
            mask_sb = const.tile([KCB, KCB], bf16, name="mask", tag="mask")
            q_sb = [const.tile([128, T], bf16, name=f"q{h}", tag=f"q{h}") for h in range(NH)]
            k_sb = [const.tile([128, T], bf16, name=f"k{h}", tag=f"k{h}") for h in range(NH)]
            v_sb = const.tile([128, (T // KCB) * VSTR], bf16, name="v", tag="v")
            yn_sb = [const.tile([128, T], bf16, name=f"yn{i}", tag=f"yn{i}") for i in range(3)]

            # input DMAs, ordered by first use
            bqk_sb = const.tile([128, 6], f32, name="bqk", tag="bqk")
            nc.sync.dma_start(out=bqk_sb, in_=d_bqk.ap())
            nc.sync.dma_start(out=mask_sb, in_=d_mk.ap())
            # first-needed tiles split across two queues each
            nc.sync.dma_start(out=xt_sb[0][0:64, :], in_=d_xt.ap()[0:64, :])
            nc.sync.dma_start(out=xt_sb[0][64:128, :], in_=d_xt.ap()[64:128, :])
            nc.sync.dma_start(out=wqk_sb[0][0:64, :], in_=d_wqk.ap()[0:64, :])
            nc.sync.dma_start(out=wqk_sb[0][64:128, :], in_=d_wqk.ap()[64:128, :])
            bvb_sb = const.tile([128, CQ], f32, name="bvb", tag="bvb")
            nc.sync.dma_start(
                out=bvb_sb,
                in_=bass.AP(tensor=d_bv, offset=0, ap=[[0, 128], [1, CQ]]))
            for i in range(1, 6):
                nc.sync.dma_start(out=xt_sb[i], in_=d_xt.ap()[128 * i:128 * (i + 1), :])
                nc.sync.dma_start(out=wqk_sb[i], in_=d_wqk.ap()[128 * i:128 * (i + 1), :])
            for i in range(NCC):
                nc.sync.dma_start(out=wv_sb[i], in_=d_wv.ap()[128 * i:128 * (i + 1), :])
            for i in range(3):
                nc.sync.dma_start(out=wp_sb[i], in_=d_wp.ap()[128 * i:128 * (i + 1), :])

            # zero the unused head-half of each padded Q^T/K^T tile
            for h in range(NH):
                dead = slice(64, 128) if h % 2 == 0 else slice(0, 64)
                nc.gpsimd.memset(q_sb[h][dead, :], 0.0)
                nc.gpsimd.memset(k_sb[h][dead, :], 0.0)
            # ones column per (key block, head) in v_sb
            v_ones = v_sb.rearrange("p (kc h e) -> p kc h e", h=NH, e=HD + 1)[:, :, :, HD:HD + 1]
            nc.gpsimd.memset(v_ones, 1.0)

            # shared pools: one PSUM work pool (qkv/v/s share 2x2-bank slots),
            # y + proj pools coexist => no phase barriers (8 banks total)
            with tc.tile_pool(name="work", bufs=2, space="PSUM") as work, \
                 tc.tile_pool(name="ps_s", bufs=2, space="PSUM") as ps_s, \
                 tc.tile_pool(name="ps_y", bufs=2, space="PSUM") as ps_y, \
                 tc.tile_pool(name="pp", bufs=4) as pp, \
                 tc.tile_pool(name="smalls", bufs=4) as smalls, \
                 tc.tile_pool(name="outp", bufs=4) as outp:

                def emit_qkv_jc(jc):
                    # feature chunk jc: 0-2 Q, 3-5 K (128 wide, 2 heads);
                    # bias added during the PSUM->SBUF copy (per-partition scalar)
                    for t4 in range(NQC):
                        ps = work.tile([128, TQ], f32, name="qkv", tag="w")
                        for cc in range(6):
                            nc.tensor.matmul(
                                ps[:, 0:TQ],
                                lhsT=wqk_sb[cc][:, 128 * jc:128 * (jc + 1)],
                                rhs=xt_sb[cc][:, TQ * t4:TQ * (t4 + 1)],
                                start=(cc == 0), stop=(cc == 5),
                            )
                        arr = q_sb if jc < 3 else k_sb
                        j = jc if jc < 3 else jc - 3
                        cols = slice(TQ * t4, TQ * (t4 + 1))
                        nc.vector.tensor_scalar_add(
                            arr[2 * j][0:64, cols], ps[0:64, 0:TQ],
                            bqk_sb[0:64, jc:jc + 1])
                        nc.vector.tensor_scalar_add(
                            arr[2 * j + 1][64:128, cols], ps[64:128, 0:TQ],
                            bqk_sb[64:128, jc:jc + 1])

                def emit_v():
                    for kc in range(T // KCB):
                        psv = work.tile([128, TQ], f32, name="psv", tag="w")
                        for cc in range(NCC):
                            nc.tensor.matmul(
                                psv[:, 0:CQ],
                                lhsT=xt_sb[cc][:, 128 * kc:128 * (kc + 1)],
                                rhs=wv_sb[cc],
                                start=(cc == 0), stop=(cc == NCC - 1),
                            )
                        dst = v_sb[:, VSTR * kc:VSTR * (kc + 1)].rearrange(
                            "p (h e) -> p h e", e=HD + 1)[:, :, 0:HD]
                        nc.vector.scalar_tensor_tensor(
                            out=dst,
                            in0=psv[:, 0:CQ].rearrange("p (h e) -> p h e", e=HD),
                            scalar=0.0,
                            in1=bvb_sb.rearrange("p (h e) -> p h e", e=HD),
                            op0=mybir.AluOpType.add, op1=mybir.AluOpType.add)

                def emit_attn_unit(h, qi):
                    q0 = TQ * qi
                    nkc = (q0 + TQ) // KCB  # causal: key blocks 0..nkc-1
                    y = ps_y.tile([HD + 1, TQ], f32, name="y", tag="y")
                    pend = []  # attV lags one S-group behind (sw pipeline)

                    def emit_attv(p, kcs):
                        for idx, kc in enumerate(kcs):
                            r = kc - 4 * qi
                            off = KCB * r if r >= 0 else 0
                            nc.tensor.matmul(
                                y[:, off:TQ],
                                lhsT=v_sb[:, VSTR * kc + (HD + 1) * h:
                                          VSTR * kc + (HD + 1) * (h + 1)],
                                rhs=p[:, TQ * idx + off:TQ * (idx + 1)],
                                start=(kc == 0), stop=(kc == nkc - 1),
                            )

                    for g0 in range(0, nkc, SGRP):
                        kcs = range(g0, min(g0 + SGRP, nkc))
                        width = len(kcs) * TQ
                        s = ps_s.tile([128, SGRP * TQ], f32, name="s", tag="s")
                        for idx, kc in enumerate(kcs):
                            nc.tensor.matmul(
                                s[:, TQ * idx:TQ * (idx + 1)],
                                lhsT=k_sb[h][:, KCB * kc:KCB * (kc + 1)],
                                rhs=q_sb[h][:, q0:q0 + TQ],
                                start=True, stop=True,
                            )
                        r0 = kcs[0] - 4 * qi
                        a0 = KCB * r0 if r0 > 0 else 0
                        p = pp.tile([128, SGRP * TQ], bf16, name="p", tag="p")
                        nc.scalar.activation(p[:, a0:width], s[:, a0:width], EXP)
                        for idx, kc in enumerate(kcs):
                            r = kc - 4 * qi
                            if r >= 0:  # diagonal block: apply triangle mask
                                lo = TQ * idx + KCB * r
                                nc.gpsimd.tensor_mul(
                                    p[:, lo:lo + KCB], p[:, lo:lo + KCB], mask_sb)
                        pend.append((p, kcs))
                        if len(pend) > 1:
                            emit_attv(*pend.pop(0))
                    emit_attv(*pend.pop(0))
                    # normalize by softmax denominator (row 64 of y)
                    rc = smalls.tile([HD + 1, TQ], f32, name="rc", tag="rc")
                    nc.vector.tensor_copy(rc[HD:HD + 1, :], y[HD:HD + 1, :])
                    rb = smalls.tile([HD, TQ], f32, name="rb", tag="rb")
                    lsrc = rc[HD:HD + 1, :]
                    for qd in range(4):  # 4 queues: descriptor-latency /4
                        nc.sync.dma_start(
                            out=rb[16 * qd:16 * (qd + 1), :],
                            in_=bass.AP(tensor=lsrc.tensor, offset=lsrc.offset,
                                        ap=[lsrc.ap[0], [0, 16], lsrc.ap[1]]))
                    nc.vector.reciprocal_approx_fast(out=rb, in_=rb)
                    fc, half = h // 2, h % 2
                    if half == 0:
                        nc.vector.tensor_mul(
                            yn_sb[fc][0:HD, q0:q0 + TQ], y[0:HD, :], rb)
                    else:
                        tt = smalls.tile([HD, TQ], bf16, name="tt", tag="tt")
                        nc.vector.tensor_mul(tt, y[0:HD, :], rb)
                        nc.sync.dma_start(
                            out=yn_sb[fc][HD:2 * HD, q0:q0 + TQ], in_=tt)

                # interleaved emission: per head-pair QKV, then its attention
                emit_qkv_jc(0)
                emit_qkv_jc(3)
                emit_v()
                def emit_proj(tcb):
                    ob = outp.tile([128, C], bf16, name="ob", tag="ob")
                    for oc in range(2):
                        po = work.tile([128, TQ], f32, name="po", tag="w")
                        for fcc in range(3):
                            nc.tensor.matmul(
                                po[:, 0:CQ],
                                lhsT=yn_sb[fcc][:, 128 * tcb:128 * (tcb + 1)],
                                rhs=wp_sb[fcc][:, CQ * oc:CQ * (oc + 1)],
                                start=(fcc == 0), stop=(fcc == 2),
                            )
                        nc.vector.tensor_copy(ob[:, CQ * oc:CQ * (oc + 1)], po[:, 0:CQ])
                    if tcb >= 12:  # trim the kernel tail: split last stores
                        nc.sync.dma_start(
                            out=d_out.ap()[128 * tcb:128 * tcb + 64, :],
                            in_=ob[0:64, :])
                        nc.sync.dma_start(
                            out=d_out.ap()[128 * tcb + 64:128 * (tcb + 1), :],
                            in_=ob[64:128, :])
                    else:
                        nc.sync.dma_start(
                            out=d_out.ap()[128 * tcb:128 * (tcb + 1), :], in_=ob)

                for pair in range(2):
                    for h in (2 * pair, 2 * pair + 1):
                        for qi in range(NQC):
                            emit_attn_unit(h, qi)
                    emit_qkv_jc(pair + 1)
                    emit_qkv_jc(pair + 4)
                for qi in range(NQC):
                    for h in (4, 5):
                        emit_attn_unit(h, qi)
                    for tcb in range(4 * qi, 4 * qi + 4):
                        emit_proj(tcb)

    nc.compile()
    return nc


def _prep_core(x, w_attn, b_attn, w_proj, c):
    b, g = c // 2, c % 2
    h0 = NH * g
    q = slice(64 * h0, 64 * h0 + CQ)
    k = slice(C + 64 * h0, C + 64 * h0 + CQ)
    v = slice(2 * C + 64 * h0, 2 * C + 64 * h0 + CQ)

    xt = np.ascontiguousarray(x[b].T).astype(_BF16)

    wqk = np.empty((CPAD, 2 * CQ), dtype=_BF16)
    wqk[:, 0:CQ] = (w_attn[:, q] * 0.125).astype(_BF16)
    wqk[:, CQ:] = w_attn[:, k].astype(_BF16)
    bqk = np.concatenate([b_attn[q] * 0.125, b_attn[k]]).astype(np.float32)
    bqk = np.ascontiguousarray(bqk.reshape(6, 128).T)

    wv = np.ascontiguousarray(w_attn[:, v]).astype(_BF16)
    bv = np.ascontiguousarray(b_attn[v].reshape(1, CQ)).astype(np.float32)

    wp = np.ascontiguousarray(w_proj[q, :]).astype(_BF16)

    ii = np.arange(KCB)
    mk = (ii[:, None] <= ii[None, :]).astype(_BF16)
    return {"xt": xt, "wqk": wqk, "wv": wv, "wp": wp, "mk": mk, "bqk": bqk,
            "bv": bv}


def kernel(x, w_attn, b_attn, w_proj, b_proj):
    from concourse.bass_utils import run_bass_kernel_spmd

    x = np.asarray(x, dtype=np.float32)
    w_attn = np.asarray(w_attn, dtype=np.float32)
    b_attn = np.asarray(b_attn, dtype=np.float32)
    w_proj = np.asarray(w_proj, dtype=np.float32)
    b_proj = np.asarray(b_proj, dtype=np.float32)

    if "nc" not in _cache:
        _cache["nc"] = _build()
    nc = _cache["nc"]

    in_maps = [_prep_core(x, w_attn, b_attn, w_proj, c) for c in range(NCORES)]
    res = run_bass_kernel_spmd(nc, in_maps, core_ids=list(range(NCORES)))

    out = np.empty((B, T, C), dtype=np.float32)
    for b in range(B):
        out[b] = (res.results[2 * b]["out"].astype(np.float32)
                  + res.results[2 * b + 1]["out"].astype(np.float32) + b_proj)
    return out

